# revision 26
# baseline (speedup 1.0000x reference)
"""Distributed multi-head attention kernel for 8 TRN2 NeuronCores.

Problem: B=2, N=2048, C=1024, H=16 heads, D=64.
  out = softmax((q@Wq)(k@Wk)^T / sqrt(D)) @ (v@Wv) @ Wo   (per head, biases are zero)

Sharding: sequence-parallel within batch (2 batch groups x 4 cores), with NO
collectives: measured AllGather on this fleet has a ~65us fixed cost plus slow
streaming (~110-145us total), while recomputing the full-batch K/V projections
locally costs ~109us of perfectly parallel PE time and keeps every core
independent.  Core c owns batch b=c//4, query rows R=[512r, 512r+512), r=c%4.

Per-core dataflow (all PE inputs bf16, PSUM/softmax f32).  ScalarE exp (~147us
total) is the scarce second resource after the PE, so S^T = K@Q^T work is
spread across every phase instead of serializing behind the projections:
  1. Q^T = Wq^T @ xq^T  (own rows; channels on partitions; stays in SBUF)
  2. K^T = Wk^T @ xk^T  full batch -> resident SBUF, S^T(pair 0) interleaved
     (exp folds the 1/8 scale; no max-subtraction needed for ~N(0,1) scores)
  3. V' = [xv @ Wv | ones] full batch -> resident SBUF, S^T(pairs 1-4)
     interleaved; exp'd P tiles for pairs 2-4 spill to DRAM via DMA
  4. Tail per pair i: O'^T(i) = V'^T @ P^T accumulated over key chunks
     (row 64 = softmax denominator via the ones column).  Exactly one P-pair
     producer runs per period: DMA reloads of pairs 2-4, then S^T of pairs
     5-7 two pairs ahead.  Normalize with fast reciprocal + partition
     broadcast into A^T.
  5. out^T = Wo^T @ A^T -> DRAM (f32). Host transposes + concatenates.
"""

import sys

sys.path.insert(0, "/opt/trn_rl_repo")

from contextlib import ExitStack

import numpy as np
import ml_dtypes

import concourse.bass as bass
import concourse.bacc as bacc
import concourse.mybir as mybir
import concourse.tile as tile
from concourse.bass_utils import run_bass_kernel_spmd

BF16 = mybir.dt.bfloat16
F32 = mybir.dt.float32
Exp = mybir.ActivationFunctionType.Exp

B, N, C = 2, 2048, 1024
H, D = 16, 64
DV = D + 1          # V columns per head incl. ones column
NQ = N // 4         # queries per core = 512
NCHUNK = N // 128   # 16 key chunks
NG = NCHUNK // 2    # 8 P groups per pair
SCALE = 1.0 / np.sqrt(D)
SPILLED = (2, 3, 4)

_CACHE = {}


def build_nc():
    nc = bacc.Bacc("TRN2", target_bir_lowering=False, debug=False, num_devices=8)

    xqT = nc.declare_dram_parameter("xqT", [C, NQ], BF16, isOutput=False)
    xkT = nc.declare_dram_parameter("xkT", [C, N], BF16, isOutput=False)
    xvT = nc.declare_dram_parameter("xvT", [C, N], BF16, isOutput=False)
    wq = nc.declare_dram_parameter("wq", [C, C], BF16, isOutput=False)
    wk = nc.declare_dram_parameter("wk", [C, C], BF16, isOutput=False)
    wv = nc.declare_dram_parameter("wv", [C, C], BF16, isOutput=False)
    wo = nc.declare_dram_parameter("wo", [C, C], BF16, isOutput=False)
    outT = nc.declare_dram_parameter("outT", [C, NQ], F32, isOutput=True)

    spill = nc.dram_tensor("spillP", [len(SPILLED) * NG, 128, 2048], BF16)

    with tile.TileContext(nc) as tc, ExitStack() as top:
        # ---------------- resident SBUF ----------------
        res = top.enter_context(tc.tile_pool(name="res", bufs=1))
        qT_sb = res.tile([128, 8 * NQ], BF16, tag="qT")     # Q^T: pair i at cols 512i
        kT_sb = res.tile([128, 8 * N], BF16, tag="kT")      # K^T: pair i at cols 2048i
        v1_sb = res.tile([128, NCHUNK * H * DV], BF16, tag="v1")  # V' chunk at 1040*kc
        aT_sb = res.tile([128, 8 * NQ], BF16, tag="aT")     # A^T accum
        dinv_sb = res.tile([64, NQ], F32, tag="dinv")
        drow_sb = res.tile([1, NQ], F32, tag="drow")
        draw_sb = res.tile([1, NQ], F32, tag="draw")

        attn_stack = ExitStack()
        P_pool = attn_stack.enter_context(tc.tile_pool(name="P_pool", bufs=16))
        sp_pool = attn_stack.enter_context(tc.tile_pool(name="sp_pool", bufs=3))

        # (pair, group) -> (128, 2048) bf16 tile holding exp'd chunks 2g, 2g+1
        P_tiles = {}

        def st_chunk(spool, i, kc):
            """S^T matmuls for (pair i, key chunk kc); per-chunk exp; spilled
            pairs stream their P groups to DRAM."""
            st = spool.tile([128, 1024], F32, tag="st", name=f"st_{i}_{kc}")
            key_sl = kT_sb[:, N * i + 128 * kc:N * i + 128 * (kc + 1)]
            nc.tensor.matmul(st[:, 0:512],
                             key_sl[0:64, :],
                             qT_sb[0:64, NQ * i:NQ * (i + 1)],
                             start=True, stop=True)
            nc.tensor.matmul(st[:, 512:1024],
                             key_sl[64:128, :],
                             qT_sb[64:128, NQ * i:NQ * (i + 1)],
                             start=True, stop=True)
            g = kc // 2
            spilled = i in SPILLED
            if kc % 2 == 0:
                pool = sp_pool if spilled else P_pool
                P_tiles[(i, g)] = pool.tile([128, 2048], BF16,
                                            tag="sp" if spilled else "P",
                                            name=f"P_{i}_{g}")
            nc.scalar.activation(
                P_tiles[(i, g)][:, 1024 * (kc % 2):1024 * (kc % 2 + 1)], st[:],
                Exp, scale=float(SCALE))
            if kc % 2 == 1 and spilled:
                si = SPILLED.index(i)
                nc.sync.dma_start(out=spill[NG * si + g], in_=P_tiles[(i, g)][:])
                del P_tiles[(i, g)]

        with ExitStack() as ph:
            wpool = ph.enter_context(tc.tile_pool(name="wpool", bufs=8))
            xqpool = ph.enter_context(tc.tile_pool(name="xqpool", bufs=8))
            xkpool = ph.enter_context(tc.tile_pool(name="xkpool", bufs=10))
            ppool = ph.enter_context(tc.tile_pool(name="ppool", bufs=6, space="PSUM"))
            spoolA = ph.enter_context(tc.tile_pool(name="spoolA", bufs=1,
                                                   space="PSUM"))

            # ---------------- Q^T projection (cc-streamed) ----------------
            xq_t, wq_t = [], []
            for cc in range(8):
                xq_t.append(xqpool.tile([128, NQ], BF16, tag="xq", name=f"xq_t{cc}"))
                nc.sync.dma_start(out=xq_t[cc][:], in_=xqT[128 * cc:128 * (cc + 1), :])
                wq_t.append(wpool.tile([128, C], BF16, tag="w", name=f"wq_t{cc}"))
                nc.sync.dma_start(out=wq_t[cc][:], in_=wq[128 * cc:128 * (cc + 1), :])
            for mg in range(2):
                ps = [ppool.tile([128, NQ], F32, tag="ps", name=f"qps{mg}_{m}")
                      for m in range(4)]
                for cc in range(8):
                    for m in range(4):
                        nc.tensor.matmul(ps[m][:],
                                         wq_t[cc][:, 128 * (4 * mg + m):
                                                   128 * (4 * mg + m + 1)],
                                         xq_t[cc][:],
                                         start=(cc == 0), stop=(cc == 7))
                for m in range(4):
                    nc.vector.tensor_copy(
                        qT_sb[:, NQ * (4 * mg + m):NQ * (4 * mg + m + 1)], ps[m][:])

            # ------- K^T projection (full batch) with S^T(0) interleaved -------
            # two passes over query halves; x^T staged as (128,1024) tiles
            wk_t = []
            for cc in range(8):
                wk_t.append(wpool.tile([128, C], BF16, tag="w", name=f"wk_t{cc}"))
                nc.sync.dma_start(out=wk_t[cc][:], in_=wk[128 * cc:128 * (cc + 1), :])
            xk_t = {}
            for hb in range(2):
                for cc in range(8):
                    t = xkpool.tile([128, 1024], BF16, tag="xk",
                                    name=f"xk_t{hb}_{cc}")
                    nc.sync.dma_start(
                        out=t[:],
                        in_=xkT[128 * cc:128 * (cc + 1),
                                1024 * hb:1024 * (hb + 1)])
                    xk_t[(hb, cc)] = t
                for m in range(8):
                    ps = [ppool.tile([128, 512], F32, tag="ps",
                                     name=f"kps{hb}_{m}_{q2}") for q2 in range(2)]
                    for cc in range(8):
                        for q2 in range(2):
                            nc.tensor.matmul(
                                ps[q2][:],
                                wk_t[cc][:, 128 * m:128 * (m + 1)],
                                xk_t[(hb, cc)][:, 512 * q2:512 * (q2 + 1)],
                                start=(cc == 0), stop=(cc == 7))
                    for q2 in range(2):
                        qb = 2 * hb + q2
                        nc.vector.tensor_copy(
                            kT_sb[:, N * m + 512 * qb:N * m + 512 * (qb + 1)],
                            ps[q2][:])
                    if hb == 1 and m >= 1:
                        for kc in range((m - 1) * 16 // 7, m * 16 // 7):
                            st_chunk(spoolA, 0, kc)

        # --- V' projection (full batch) with S^T(pairs 1-4) interleaved ---
        with ExitStack() as ph:
            wvpool = ph.enter_context(tc.tile_pool(name="wvpool", bufs=8))
            xvpool = ph.enter_context(tc.tile_pool(name="xvpool", bufs=10))
            vppool = ph.enter_context(tc.tile_pool(name="vppool", bufs=2, space="PSUM"))
            spoolB = ph.enter_context(tc.tile_pool(name="spoolB", bufs=2,
                                                   space="PSUM"))
            wv_t = []
            for cc in range(8):
                wv_t.append(wvpool.tile([128, C], BF16, tag="w", name=f"wv_t{cc}"))
                nc.sync.dma_start(out=wv_t[cc][:], in_=wv[128 * cc:128 * (cc + 1), :])
            v3 = v1_sb[:].rearrange("p (kc h x) -> p kc h x", kc=NCHUNK, x=DV)
            nc.vector.memset(v3[:, :, :, D:DV], 1.0)
            xv_t = {}
            for kc in range(NCHUNK):
                half_blk, sub = kc // 8, kc % 8
                if sub == 0:
                    for cc in range(8):
                        t = xvpool.tile([128, 1024], BF16, tag="xv",
                                        name=f"xv_t{half_blk}_{cc}")
                        nc.sync.dma_start(
                            out=t[:],
                            in_=xvT[128 * cc:128 * (cc + 1),
                                    1024 * half_blk:1024 * (half_blk + 1)])
                        xv_t[(half_blk, cc)] = t
                ps = vppool.tile([128, 1024], F32, tag="vp", name=f"vps{kc}")
                for cc in range(8):
                    for half in range(2):
                        nc.tensor.matmul(
                            ps[:, 512 * half:512 * (half + 1)],
                            xv_t[(half_blk, cc)][:, 128 * sub:128 * (sub + 1)],
                            wv_t[cc][:, 512 * half:512 * (half + 1)],
                            start=(cc == 0), stop=(cc == 7))
                for half in range(2):
                    nc.vector.tensor_copy(
                        v3[:, kc, 8 * half:8 * (half + 1), 0:D],
                        ps[:, 512 * half:512 * (half + 1)]
                        .rearrange("p (h d) -> p h d", d=D))
                for sp in range(4):   # pairs 1-4, chunk kc
                    st_chunk(spoolB, 1 + sp, kc)

        # ---- tail: PV(i) with one P-producer per period ----
        with ExitStack() as ph:
            opool = ph.enter_context(tc.tile_pool(name="opool", bufs=2, space="PSUM"))
            spoolC = ph.enter_context(tc.tile_pool(name="spoolC", bufs=3,
                                                   space="PSUM"))
            wopool = ph.enter_context(tc.tile_pool(name="wopool", bufs=8))
            wo_t = [wopool.tile([128, C], BF16, tag="wo", name=f"wo_t{j}")
                    for j in range(8)]
            for cc in range(8):
                nc.sync.dma_start(out=wo_t[cc][:], in_=wo[128 * cc:128 * (cc + 1), :])
            v4 = v1_sb[:].rearrange("p (kc v) -> p kc v", v=H * DV)
            for i in range(8):
                # reload spilled P for pair i+2 (consumes slots PV(i) frees)
                if i + 2 in SPILLED:
                    si = SPILLED.index(i + 2)
                    for g in range(NG):
                        rl = P_pool.tile([128, 2048], BF16, tag="P",
                                         name=f"rl_{i + 2}_{g}")
                        nc.sync.dma_start(out=rl[:], in_=spill[NG * si + g])
                        P_tiles[(i + 2, g)] = rl
                po = [opool.tile([128, NQ], F32, tag="po", name=f"po{i}_{h}")
                      for h in range(2)]
                for kc in range(NCHUNK):
                    if 3 <= i <= 5:   # S^T for pairs 5-7, two pairs ahead
                        st_chunk(spoolC, i + 2, kc)
                    for h in range(2):
                        nc.tensor.matmul(
                            po[h][0:DV, :],
                            v4[:, kc, 2 * DV * i + DV * h:2 * DV * i + DV * (h + 1)],
                            P_tiles[(i, kc // 2)]
                            [:, 1024 * (kc % 2) + 512 * h:
                             1024 * (kc % 2) + 512 * h + 512],
                            start=(kc == 0), stop=(kc == NCHUNK - 1))
                for h in range(2):
                    nc.vector.tensor_copy(draw_sb[:], po[h][D:DV, :])
                    nc.vector.reciprocal_approx_fast(drow_sb[:], draw_sb[:])
                    nc.gpsimd.partition_broadcast(dinv_sb[:], drow_sb[:])
                    nc.vector.tensor_mul(
                        aT_sb[64 * h:64 * (h + 1), NQ * i:NQ * (i + 1)],
                        po[h][0:D, :], dinv_sb[:])
                for g in range(NG):
                    del P_tiles[(i, g)]

            # ---------------- output projection ----------------
            epool = ph.enter_context(tc.tile_pool(name="eopool", bufs=3))
            for m in range(8):
                ps = opool.tile([128, NQ], F32, tag="po", name=f"ops{m}")
                for cc in range(8):
                    nc.tensor.matmul(ps[:], wo_t[cc][:, 128 * m:128 * (m + 1)],
                                     aT_sb[:, NQ * cc:NQ * (cc + 1)],
                                     start=(cc == 0), stop=(cc == 7))
                ev = epool.tile([128, NQ], F32, tag="ev", name=f"oev{m}")
                nc.vector.tensor_copy(ev[:], ps[:])
                nc.sync.dma_start(out=outT[128 * m:128 * (m + 1), :], in_=ev[:])
        attn_stack.close()

    nc.compile()
    return nc


def _get_nc():
    if "nc" not in _CACHE:
        _CACHE["nc"] = build_nc()
    return _CACHE["nc"]


def _make_in_maps(q, k, v, Wq, Wk, Wv, Wo):
    bf = ml_dtypes.bfloat16
    wq_b = np.ascontiguousarray(Wq).astype(bf)
    wk_b = np.ascontiguousarray(Wk).astype(bf)
    wv_b = np.ascontiguousarray(Wv).astype(bf)
    wo_b = np.ascontiguousarray(Wo).astype(bf)
    q = np.asarray(q)
    kT = [np.ascontiguousarray(np.asarray(k)[b].T).astype(bf) for b in range(B)]
    vT = [np.ascontiguousarray(np.asarray(v)[b].T).astype(bf) for b in range(B)]
    in_maps = []
    for c in range(8):
        b, r = c // 4, c % 4
        sl = slice(NQ * r, NQ * (r + 1))
        in_maps.append({
            "xqT": np.ascontiguousarray(q[b, sl, :].T).astype(bf),
            "xkT": kT[b], "xvT": vT[b],
            "wq": wq_b, "wk": wk_b, "wv": wv_b, "wo": wo_b,
        })
    return in_maps


def _run(inputs, trace=False, **kw):
    nc = _get_nc()
    in_maps = _make_in_maps(inputs["q"], inputs["k"], inputs["v"],
                            inputs["Wq"], inputs["Wk"], inputs["Wv"], inputs["Wo"])
    res = run_bass_kernel_spmd(nc, in_maps, core_ids=list(range(8)), trace=trace, **kw)
    out = np.empty((B, N, C), np.float32)
    for c in range(8):
        b, r = c // 4, c % 4
        out[b, NQ * r:NQ * (r + 1), :] = res.results[c]["outT"].T
    return out, res


def kernel(**inputs) -> np.ndarray:
    out, _ = _run(inputs, trace=False)
    return out


# revision 31
# speedup vs baseline: 1.0213x; 1.0213x over previous
"""Distributed multi-head attention kernel for 8 TRN2 NeuronCores.

Problem: B=2, N=2048, C=1024, H=16 heads, D=64.
  out = softmax((q@Wq)(k@Wk)^T / sqrt(D)) @ (v@Wv) @ Wo   (per head, biases are zero)

Sharding: sequence-parallel within batch (2 batch groups x 4 cores), with NO
collectives: measured AllGather on this fleet has a ~65us fixed cost plus slow
streaming (~110-145us total), while recomputing the full-batch K/V projections
locally costs ~109us of perfectly parallel PE time and keeps every core
independent.  Core c owns batch b=c//4, query rows R=[512r, 512r+512), r=c%4.

Per-core dataflow (all PE inputs bf16, PSUM/softmax f32):
  1. Q^T = Wq^T @ xq^T  (own rows; channels on partitions; stays in SBUF)
  2. K^T = Wk^T @ xk^T  for the FULL batch -> resident SBUF (128, 8*2048)
     S^T(pair i) = K @ Q^T interleaved right after K^T; exp on ScalarE
     (scale=1/8 folded in; no max-subtraction needed for ~N(0,1) scores)
  3. V' = [xv @ Wv | ones] full batch, per-head 65-col groups -> resident SBUF
  4. O'^T = V'^T @ P^T accumulated over key chunks; row 64 = softmax denom.
     Normalize with reciprocal broadcast; accumulate A^T in SBUF.
     PV(i) is software-pipelined against S^T(i+1) so the PE never waits on exp.
  5. out^T = Wo^T @ A^T -> DRAM (f32). Host transposes + concatenates.
"""

import sys

sys.path.insert(0, "/opt/trn_rl_repo")

from contextlib import ExitStack

import numpy as np
import ml_dtypes

import concourse.bass as bass
import concourse.bacc as bacc
import concourse.mybir as mybir
import concourse.tile as tile
from concourse.bass_utils import run_bass_kernel_spmd

BF16 = mybir.dt.bfloat16
F32 = mybir.dt.float32
Exp = mybir.ActivationFunctionType.Exp

B, N, C = 2, 2048, 1024
H, D = 16, 64
DV = D + 1          # V columns per head incl. ones column
NQ = N // 4         # queries per core = 512
NCHUNK = N // 128   # 16 key chunks
SCALE = 1.0 / np.sqrt(D)
PRELUDE = 1         # S^T pairs emitted before the V' projection phase

_CACHE = {}


def build_nc():
    nc = bacc.Bacc("TRN2", target_bir_lowering=False, debug=False, num_devices=8)

    xqT = nc.declare_dram_parameter("xqT", [C, NQ], BF16, isOutput=False)
    xkT = nc.declare_dram_parameter("xkT", [C, N], BF16, isOutput=False)
    xvT = nc.declare_dram_parameter("xvT", [C, N], BF16, isOutput=False)
    wq = nc.declare_dram_parameter("wq", [C, C], BF16, isOutput=False)
    wk = nc.declare_dram_parameter("wk", [C, C], BF16, isOutput=False)
    wv = nc.declare_dram_parameter("wv", [C, C], BF16, isOutput=False)
    wo = nc.declare_dram_parameter("wo", [C, C], BF16, isOutput=False)
    outT = nc.declare_dram_parameter("outT", [C, NQ], F32, isOutput=True)

    with tile.TileContext(nc) as tc, ExitStack() as top:
        # ---------------- resident SBUF (~83 KB/partition) ----------------
        res = top.enter_context(tc.tile_pool(name="res", bufs=1))
        qT_sb = res.tile([128, 8 * NQ], BF16, tag="qT")     # Q^T: pair i at cols 512i
        kT_sb = res.tile([128, 8 * N], BF16, tag="kT")      # K^T: pair i at cols 2048i
        v1_sb = res.tile([128, NCHUNK * H * DV], BF16, tag="v1")  # V' chunk kc at 1040*kc
        aT_sb = res.tile([128, 8 * NQ], BF16, tag="aT")     # A^T accum
        dinv_sb = res.tile([64, NQ], F32, tag="dinv")
        drow_sb = res.tile([1, NQ], F32, tag="drow")
        draw_sb = res.tile([1, NQ], F32, tag="draw")

        attn_stack = ExitStack()

        # (pair, group) -> (128, 2048) bf16 tile holding exp'd chunks 2g, 2g+1
        P_tiles = {}

        def st_chunk(i, kc):
            """S^T matmuls + per-chunk exp for (pair i, key chunk kc)."""
            st = spool.tile([128, 1024], F32, tag="st", name=f"st_{i}_{kc}")
            key_sl = kT_sb[:, N * i + 128 * kc:N * i + 128 * (kc + 1)]
            nc.tensor.matmul(st[:, 0:512],
                             key_sl[0:64, :],
                             qT_sb[0:64, NQ * i:NQ * (i + 1)],
                             start=True, stop=True)
            nc.tensor.matmul(st[:, 512:1024],
                             key_sl[64:128, :],
                             qT_sb[64:128, NQ * i:NQ * (i + 1)],
                             start=True, stop=True)
            g = kc // 2
            if kc % 2 == 0:
                P_tiles[(i, g)] = P_pool.tile([128, 2048], BF16, tag="P",
                                              name=f"P_{i}_{g}")
            nc.scalar.activation(
                P_tiles[(i, g)][:, 1024 * (kc % 2):1024 * (kc % 2 + 1)], st[:],
                Exp, scale=float(SCALE))

        with ExitStack() as ph:
            wpool = ph.enter_context(tc.tile_pool(name="wpool", bufs=9))
            xqpool = ph.enter_context(tc.tile_pool(name="xqpool", bufs=8))
            xkpool = ph.enter_context(tc.tile_pool(name="xkpool", bufs=8))
            ppool = ph.enter_context(tc.tile_pool(name="ppool", bufs=8, space="PSUM"))

            # ---------------- Q^T projection ----------------
            # cc-streamed: psum[m 0-3] and [4-7] accumulate as (wq[cc], xq[cc])
            # arrive, so the first matmul only waits on the cc=0 DMAs.
            xq_t, wq_t = [], []
            for cc in range(8):
                xq_t.append(xqpool.tile([128, NQ], BF16, tag="xq", name=f"xq_t{cc}"))
                nc.sync.dma_start(out=xq_t[cc][:], in_=xqT[128 * cc:128 * (cc + 1), :])
                wq_t.append(wpool.tile([128, C], BF16, tag="w", name=f"wq_t{cc}"))
                nc.sync.dma_start(out=wq_t[cc][:], in_=wq[128 * cc:128 * (cc + 1), :])
            for mg in range(2):
                ps = [ppool.tile([128, NQ], F32, tag="ps", name=f"qps{mg}_{m}")
                      for m in range(4)]
                for cc in range(8):
                    for m in range(4):
                        nc.tensor.matmul(ps[m][:],
                                         wq_t[cc][:, 128 * (4 * mg + m):
                                                   128 * (4 * mg + m + 1)],
                                         xq_t[cc][:],
                                         start=(cc == 0), stop=(cc == 7))
                for m in range(4):
                    nc.vector.tensor_copy(
                        qT_sb[:, NQ * (4 * mg + m):NQ * (4 * mg + m + 1)], ps[m][:])

            # ------------- K^T projection (full batch) -------------
            # lhsT (wk slice) reused across the 4 query blocks per ldweights.
            wk_t, xk_t = [], []
            for cc in range(8):
                wk_t.append(wpool.tile([128, C], BF16, tag="w", name=f"wk_t{cc}"))
                nc.sync.dma_start(out=wk_t[cc][:], in_=wk[128 * cc:128 * (cc + 1), :])
                xk_t.append(xkpool.tile([128, N], BF16, tag="xk", name=f"xk_t{cc}"))
                nc.sync.dma_start(out=xk_t[cc][:], in_=xkT[128 * cc:128 * (cc + 1), :])
            for m in range(8):
                ps = [ppool.tile([128, 512], F32, tag="ps", name=f"kps{m}_{qb}")
                      for qb in range(4)]
                for cc in range(8):
                    for qb in range(4):
                        nc.tensor.matmul(ps[qb][:],
                                         wk_t[cc][:, 128 * m:128 * (m + 1)],
                                         xk_t[cc][:, 512 * qb:512 * (qb + 1)],
                                         start=(cc == 0), stop=(cc == 7))
                for qb in range(4):
                    nc.vector.tensor_copy(
                        kT_sb[:, N * m + 512 * qb:N * m + 512 * (qb + 1)], ps[qb][:])

        spool = attn_stack.enter_context(
            tc.tile_pool(name="spool", bufs=3, space="PSUM"))       # 6 banks
        P_pool = attn_stack.enter_context(tc.tile_pool(name="P_pool", bufs=16))

        # ---------------- V' projection (full batch) ----------------
        # psum comes from spool (one (128,1024) tile per key chunk, both
        # halves); lhsT (xv chunk) reused across both halves per ldweights.
        with ExitStack() as ph:
            wpool = ph.enter_context(tc.tile_pool(name="wvpool", bufs=9))
            xvpool = ph.enter_context(tc.tile_pool(name="xvpool", bufs=8))
            wv_t, xv_t = [], []
            for cc in range(8):
                wv_t.append(wpool.tile([128, C], BF16, tag="w", name=f"wv_t{cc}"))
                nc.sync.dma_start(out=wv_t[cc][:], in_=wv[128 * cc:128 * (cc + 1), :])
                xv_t.append(xvpool.tile([128, N], BF16, tag="xv", name=f"xv_t{cc}"))
                nc.sync.dma_start(out=xv_t[cc][:], in_=xvT[128 * cc:128 * (cc + 1), :])
            v3 = v1_sb[:].rearrange("p (kc h x) -> p kc h x", kc=NCHUNK, x=DV)
            nc.vector.memset(v3[:, :, :, D:DV], 1.0)
            for kc in range(NCHUNK):
                ps = spool.tile([128, 1024], F32, tag="st", name=f"vps{kc}")
                for cc in range(8):
                    for half in range(2):
                        nc.tensor.matmul(
                            ps[:, 512 * half:512 * (half + 1)],
                            xv_t[cc][:, 128 * kc:128 * (kc + 1)],
                            wv_t[cc][:, 512 * half:512 * (half + 1)],
                            start=(cc == 0), stop=(cc == 7))
                for half in range(2):
                    nc.vector.tensor_copy(
                        v3[:, kc, 8 * half:8 * (half + 1), 0:D],
                        ps[:, 512 * half:512 * (half + 1)]
                        .rearrange("p (h d) -> p h d", d=D))
                st_chunk(0, kc)
                st_chunk(1, kc)

        # -- tail: S^T(i+2, kc) interleaved per chunk with PV(i, kc) --
        with ExitStack() as ph:
            opool = ph.enter_context(tc.tile_pool(name="opool", bufs=2, space="PSUM"))
            wopool = ph.enter_context(tc.tile_pool(name="wopool", bufs=8))
            wo_t = [wopool.tile([128, C], BF16, tag="wo", name=f"wo_t{j}")
                    for j in range(8)]
            for cc in range(8):
                nc.sync.dma_start(out=wo_t[cc][:], in_=wo[128 * cc:128 * (cc + 1), :])
            v4 = v1_sb[:].rearrange("p (kc v) -> p kc v", v=H * DV)
            for i in range(8):
                po = [opool.tile([128, NQ], F32, tag="po", name=f"po{i}_{h}")
                      for h in range(2)]
                for kc in range(NCHUNK):
                    if i + 2 < 8:
                        st_chunk(i + 2, kc)
                    for h in range(2):
                        nc.tensor.matmul(
                            po[h][0:DV, :],
                            v4[:, kc, 2 * DV * i + DV * h:2 * DV * i + DV * (h + 1)],
                            P_tiles[(i, kc // 2)]
                            [:, 1024 * (kc % 2) + 512 * h:
                             1024 * (kc % 2) + 512 * h + 512],
                            start=(kc == 0), stop=(kc == NCHUNK - 1))
                for h in range(2):
                    nc.vector.tensor_copy(draw_sb[:], po[h][D:DV, :])
                    nc.vector.reciprocal_approx_fast(drow_sb[:], draw_sb[:])
                    nc.gpsimd.partition_broadcast(dinv_sb[:], drow_sb[:])
                    nc.vector.tensor_mul(
                        aT_sb[64 * h:64 * (h + 1), NQ * i:NQ * (i + 1)],
                        po[h][0:D, :], dinv_sb[:])
                for g in range(NCHUNK // 2):
                    del P_tiles[(i, g)]

            # ---------------- output projection ----------------
            epool = ph.enter_context(tc.tile_pool(name="eopool", bufs=3))
            for m in range(8):
                ps = opool.tile([128, NQ], F32, tag="po", name=f"ops{m}")
                for cc in range(8):
                    nc.tensor.matmul(ps[:], wo_t[cc][:, 128 * m:128 * (m + 1)],
                                     aT_sb[:, NQ * cc:NQ * (cc + 1)],
                                     start=(cc == 0), stop=(cc == 7))
                ev = epool.tile([128, NQ], F32, tag="ev", name=f"oev{m}")
                nc.vector.tensor_copy(ev[:], ps[:])
                nc.sync.dma_start(out=outT[128 * m:128 * (m + 1), :], in_=ev[:])
        attn_stack.close()

    nc.compile()
    return nc


def _get_nc():
    if "nc" not in _CACHE:
        _CACHE["nc"] = build_nc()
    return _CACHE["nc"]


def _make_in_maps(q, k, v, Wq, Wk, Wv, Wo):
    bf = ml_dtypes.bfloat16
    wq_b = np.ascontiguousarray(Wq).astype(bf)
    wk_b = np.ascontiguousarray(Wk).astype(bf)
    wv_b = np.ascontiguousarray(Wv).astype(bf)
    wo_b = np.ascontiguousarray(Wo).astype(bf)
    q = np.asarray(q)
    kT = [np.ascontiguousarray(np.asarray(k)[b].T).astype(bf) for b in range(B)]
    vT = [np.ascontiguousarray(np.asarray(v)[b].T).astype(bf) for b in range(B)]
    in_maps = []
    for c in range(8):
        b, r = c // 4, c % 4
        sl = slice(NQ * r, NQ * (r + 1))
        in_maps.append({
            "xqT": np.ascontiguousarray(q[b, sl, :].T).astype(bf),
            "xkT": kT[b], "xvT": vT[b],
            "wq": wq_b, "wk": wk_b, "wv": wv_b, "wo": wo_b,
        })
    return in_maps


def _run(inputs, trace=False, **kw):
    nc = _get_nc()
    in_maps = _make_in_maps(inputs["q"], inputs["k"], inputs["v"],
                            inputs["Wq"], inputs["Wk"], inputs["Wv"], inputs["Wo"])
    res = None
    for attempt in range(3):
        try:
            res = run_bass_kernel_spmd(nc, in_maps, core_ids=list(range(8)),
                                       trace=trace, **kw)
            break
        except Exception:
            if attempt == 2:
                raise
            import time
            time.sleep(2.0)
    out = np.empty((B, N, C), np.float32)
    for c in range(8):
        b, r = c // 4, c % 4
        out[b, NQ * r:NQ * (r + 1), :] = res.results[c]["outT"].T
    return out, res


def kernel(**inputs) -> np.ndarray:
    out, _ = _run(inputs, trace=False)
    return out


# revision 32
# speedup vs baseline: 1.0383x; 1.0167x over previous
"""Distributed multi-head attention kernel for 8 TRN2 NeuronCores.

Problem: B=2, N=2048, C=1024, H=16 heads, D=64.
  out = softmax((q@Wq)(k@Wk)^T / sqrt(D)) @ (v@Wv) @ Wo   (per head, biases are zero)

Sharding: sequence-parallel within batch (2 batch groups x 4 cores), with NO
collectives: measured AllGather on this fleet has a ~65us fixed cost plus slow
streaming (~110-145us total), while recomputing the full-batch K/V projections
locally costs ~109us of perfectly parallel PE time and keeps every core
independent.  Core c owns batch b=c//4, query rows R=[512r, 512r+512), r=c%4.

Per-core dataflow (all PE inputs bf16, PSUM/softmax f32):
  1. Q^T = Wq^T @ xq^T  (own rows; channels on partitions; stays in SBUF)
  2. K^T = Wk^T @ xk^T  for the FULL batch -> resident SBUF (128, 8*2048)
     S^T(pair i) = K @ Q^T interleaved right after K^T; exp on ScalarE
     (scale=1/8 folded in; no max-subtraction needed for ~N(0,1) scores)
  3. V' = [xv @ Wv | ones] full batch, per-head 65-col groups -> resident SBUF
  4. O'^T = V'^T @ P^T accumulated over key chunks; row 64 = softmax denom.
     Normalize with reciprocal broadcast; accumulate A^T in SBUF.
     PV(i) is software-pipelined against S^T(i+1) so the PE never waits on exp.
  5. out^T = Wo^T @ A^T -> DRAM (f32). Host transposes + concatenates.
"""

import sys

sys.path.insert(0, "/opt/trn_rl_repo")

from contextlib import ExitStack

import numpy as np
import ml_dtypes

import concourse.bass as bass
import concourse.bacc as bacc
import concourse.mybir as mybir
import concourse.tile as tile
from concourse.bass_utils import run_bass_kernel_spmd

BF16 = mybir.dt.bfloat16
F32 = mybir.dt.float32
Exp = mybir.ActivationFunctionType.Exp

B, N, C = 2, 2048, 1024
H, D = 16, 64
DV = D + 1          # V columns per head incl. ones column
NQ = N // 4         # queries per core = 512
NCHUNK = N // 128   # 16 key chunks
SCALE = 1.0 / np.sqrt(D)
PRELUDE = 1         # S^T pairs emitted before the V' projection phase

_CACHE = {}


def build_nc():
    nc = bacc.Bacc("TRN2", target_bir_lowering=False, debug=False, num_devices=8)

    xqT = nc.declare_dram_parameter("xqT", [C, NQ], BF16, isOutput=False)
    xkT = nc.declare_dram_parameter("xkT", [C, N], BF16, isOutput=False)
    xvT = nc.declare_dram_parameter("xvT", [C, N], BF16, isOutput=False)
    wq = nc.declare_dram_parameter("wq", [C, C], BF16, isOutput=False)
    wk = nc.declare_dram_parameter("wk", [C, C], BF16, isOutput=False)
    wv = nc.declare_dram_parameter("wv", [C, C], BF16, isOutput=False)
    wo = nc.declare_dram_parameter("wo", [C, C], BF16, isOutput=False)
    outT = nc.declare_dram_parameter("outT", [C, NQ], F32, isOutput=True)

    with tile.TileContext(nc) as tc, ExitStack() as top:
        # ---------------- resident SBUF (~83 KB/partition) ----------------
        res = top.enter_context(tc.tile_pool(name="res", bufs=1))
        qT_sb = res.tile([128, 8 * NQ], BF16, tag="qT")     # Q^T: pair i at cols 512i
        kT_sb = res.tile([128, 8 * N], BF16, tag="kT")      # K^T: pair i at cols 2048i
        v1_sb = res.tile([128, NCHUNK * H * DV], BF16, tag="v1")  # V' chunk kc at 1040*kc
        aT_sb = res.tile([128, 8 * NQ], BF16, tag="aT")     # A^T accum
        dinv_sb = res.tile([64, NQ], F32, tag="dinv")
        drow_sb = res.tile([1, NQ], F32, tag="drow")
        draw_sb = res.tile([1, NQ], F32, tag="draw")

        attn_stack = ExitStack()

        def st_pair(i):
            """Emit S^T + exp for head pair i; returns the P tile."""
            Pp = P_pool.tile([128, NCHUNK * 1024], BF16, tag="P", name=f"P_{i}")
            for kc in range(NCHUNK):
                st = spool.tile([128, 1024], F32, tag="st", name=f"st_{i}_{kc}")
                key_sl = kT_sb[:, N * i + 128 * kc:N * i + 128 * (kc + 1)]
                nc.tensor.matmul(st[:, 0:512],
                                 key_sl[0:64, :],
                                 qT_sb[0:64, NQ * i:NQ * (i + 1)],
                                 start=True, stop=True)
                nc.tensor.matmul(st[:, 512:1024],
                                 key_sl[64:128, :],
                                 qT_sb[64:128, NQ * i:NQ * (i + 1)],
                                 start=True, stop=True)
                nc.scalar.activation(Pp[:, 1024 * kc:1024 * (kc + 1)], st[:],
                                     Exp, scale=float(SCALE))
            return Pp

        with ExitStack() as ph:
            wpool = ph.enter_context(tc.tile_pool(name="wpool", bufs=9))
            xqpool = ph.enter_context(tc.tile_pool(name="xqpool", bufs=8))
            xkpool = ph.enter_context(tc.tile_pool(name="xkpool", bufs=8))
            ppool = ph.enter_context(tc.tile_pool(name="ppool", bufs=8, space="PSUM"))

            # ---------------- Q^T projection ----------------
            # cc-streamed: psum[m 0-3] and [4-7] accumulate as (wq[cc], xq[cc])
            # arrive, so the first matmul only waits on the cc=0 DMAs.
            xq_t, wq_t = [], []
            for cc in range(8):
                xq_t.append(xqpool.tile([128, NQ], BF16, tag="xq", name=f"xq_t{cc}"))
                nc.sync.dma_start(out=xq_t[cc][:], in_=xqT[128 * cc:128 * (cc + 1), :])
                wq_t.append(wpool.tile([128, C], BF16, tag="w", name=f"wq_t{cc}"))
                nc.sync.dma_start(out=wq_t[cc][:], in_=wq[128 * cc:128 * (cc + 1), :])
            for mg in range(2):
                ps = [ppool.tile([128, NQ], F32, tag="ps", name=f"qps{mg}_{m}")
                      for m in range(4)]
                for cc in range(8):
                    for m in range(4):
                        nc.tensor.matmul(ps[m][:],
                                         wq_t[cc][:, 128 * (4 * mg + m):
                                                   128 * (4 * mg + m + 1)],
                                         xq_t[cc][:],
                                         start=(cc == 0), stop=(cc == 7))
                for m in range(4):
                    nc.vector.tensor_copy(
                        qT_sb[:, NQ * (4 * mg + m):NQ * (4 * mg + m + 1)], ps[m][:])

            # ------------- K^T projection (full batch) -------------
            # lhsT (wk slice) reused across the 4 query blocks per ldweights.
            wk_t, xk_t = [], []
            for cc in range(8):
                wk_t.append(wpool.tile([128, C], BF16, tag="w", name=f"wk_t{cc}"))
                nc.sync.dma_start(out=wk_t[cc][:], in_=wk[128 * cc:128 * (cc + 1), :])
                xk_t.append(xkpool.tile([128, N], BF16, tag="xk", name=f"xk_t{cc}"))
                nc.sync.dma_start(out=xk_t[cc][:], in_=xkT[128 * cc:128 * (cc + 1), :])
            for m in range(8):
                ps = [ppool.tile([128, 512], F32, tag="ps", name=f"kps{m}_{qb}")
                      for qb in range(4)]
                for cc in range(8):
                    for qb in range(4):
                        nc.tensor.matmul(ps[qb][:],
                                         wk_t[cc][:, 128 * m:128 * (m + 1)],
                                         xk_t[cc][:, 512 * qb:512 * (qb + 1)],
                                         start=(cc == 0), stop=(cc == 7))
                for qb in range(4):
                    nc.vector.tensor_copy(
                        kT_sb[:, N * m + 512 * qb:N * m + 512 * (qb + 1)], ps[qb][:])

        spool = attn_stack.enter_context(
            tc.tile_pool(name="spool", bufs=3, space="PSUM"))       # 6 banks
        P_pool = attn_stack.enter_context(
            tc.tile_pool(name="P_pool", bufs=PRELUDE + 1))          # 32 KB each

        P_tiles = {}
        for i in range(PRELUDE):
            P_tiles[i] = st_pair(i)

        # ---------------- V' projection (full batch) ----------------
        # psum comes from spool (one (128,1024) tile per key chunk, both
        # halves); lhsT (xv chunk) reused across both halves per ldweights.
        with ExitStack() as ph:
            wpool = ph.enter_context(tc.tile_pool(name="wvpool", bufs=9))
            xvpool = ph.enter_context(tc.tile_pool(name="xvpool", bufs=8))
            wv_t, xv_t = [], []
            for cc in range(8):
                wv_t.append(wpool.tile([128, C], BF16, tag="w", name=f"wv_t{cc}"))
                nc.sync.dma_start(out=wv_t[cc][:], in_=wv[128 * cc:128 * (cc + 1), :])
                xv_t.append(xvpool.tile([128, N], BF16, tag="xv", name=f"xv_t{cc}"))
                nc.sync.dma_start(out=xv_t[cc][:], in_=xvT[128 * cc:128 * (cc + 1), :])
            v3 = v1_sb[:].rearrange("p (kc h x) -> p kc h x", kc=NCHUNK, x=DV)
            nc.vector.memset(v3[:, :, :, D:DV], 1.0)
            for kc in range(NCHUNK):
                ps = spool.tile([128, 1024], F32, tag="st", name=f"vps{kc}")
                for cc in range(8):
                    for half in range(2):
                        nc.tensor.matmul(
                            ps[:, 512 * half:512 * (half + 1)],
                            xv_t[cc][:, 128 * kc:128 * (kc + 1)],
                            wv_t[cc][:, 512 * half:512 * (half + 1)],
                            start=(cc == 0), stop=(cc == 7))
                for half in range(2):
                    nc.vector.tensor_copy(
                        v3[:, kc, 8 * half:8 * (half + 1), 0:D],
                        ps[:, 512 * half:512 * (half + 1)]
                        .rearrange("p (h d) -> p h d", d=D))

        # ---------- attention: PV(i) pipelined against S^T(i+PRELUDE) ----------
        with ExitStack() as ph:
            opool = ph.enter_context(tc.tile_pool(name="opool", bufs=2, space="PSUM"))
            wopool = ph.enter_context(tc.tile_pool(name="wopool", bufs=8))
            wo_t = [wopool.tile([128, C], BF16, tag="wo", name=f"wo_t{j}")
                    for j in range(8)]
            for cc in range(8):
                nc.sync.dma_start(out=wo_t[cc][:], in_=wo[128 * cc:128 * (cc + 1), :])
            for i in range(8):
                Pp = P_tiles.pop(i)
                if i + PRELUDE < 8:
                    P_tiles[i + PRELUDE] = st_pair(i + PRELUDE)
                for h in range(2):
                    po = opool.tile([128, NQ], F32, tag="po", name=f"po{i}_{h}")
                    for kc in range(NCHUNK):
                        nc.tensor.matmul(
                            po[0:DV, :],
                            v1_sb[:].rearrange("p (kc v) -> p kc v", v=H * DV)
                                 [:, kc, 2 * DV * i + DV * h:2 * DV * i + DV * (h + 1)],
                            Pp[:, 1024 * kc + 512 * h:1024 * kc + 512 * h + 512],
                            start=(kc == 0), stop=(kc == NCHUNK - 1))
                    nc.vector.tensor_copy(draw_sb[:], po[D:DV, :])
                    nc.vector.reciprocal_approx_fast(drow_sb[:], draw_sb[:])
                    nc.gpsimd.partition_broadcast(dinv_sb[:], drow_sb[:])
                    nc.vector.tensor_mul(
                        aT_sb[64 * h:64 * (h + 1), NQ * i:NQ * (i + 1)],
                        po[0:D, :], dinv_sb[:])

            # ---------------- output projection ----------------
            epool = ph.enter_context(tc.tile_pool(name="eopool", bufs=3))
            for m in range(8):
                ps = opool.tile([128, NQ], F32, tag="po", name=f"ops{m}")
                for cc in range(8):
                    nc.tensor.matmul(ps[:], wo_t[cc][:, 128 * m:128 * (m + 1)],
                                     aT_sb[:, NQ * cc:NQ * (cc + 1)],
                                     start=(cc == 0), stop=(cc == 7))
                ev = epool.tile([128, NQ], F32, tag="ev", name=f"oev{m}")
                nc.vector.tensor_copy(ev[:], ps[:])
                nc.sync.dma_start(out=outT[128 * m:128 * (m + 1), :], in_=ev[:])
        attn_stack.close()

    nc.compile()
    return nc


def _get_nc():
    if "nc" not in _CACHE:
        _CACHE["nc"] = build_nc()
    return _CACHE["nc"]


def _make_in_maps(q, k, v, Wq, Wk, Wv, Wo):
    bf = ml_dtypes.bfloat16
    wq_b = np.ascontiguousarray(Wq).astype(bf)
    wk_b = np.ascontiguousarray(Wk).astype(bf)
    wv_b = np.ascontiguousarray(Wv).astype(bf)
    wo_b = np.ascontiguousarray(Wo).astype(bf)
    q = np.asarray(q)
    kT = [np.ascontiguousarray(np.asarray(k)[b].T).astype(bf) for b in range(B)]
    vT = [np.ascontiguousarray(np.asarray(v)[b].T).astype(bf) for b in range(B)]
    in_maps = []
    for c in range(8):
        b, r = c // 4, c % 4
        sl = slice(NQ * r, NQ * (r + 1))
        in_maps.append({
            "xqT": np.ascontiguousarray(q[b, sl, :].T).astype(bf),
            "xkT": kT[b], "xvT": vT[b],
            "wq": wq_b, "wk": wk_b, "wv": wv_b, "wo": wo_b,
        })
    return in_maps


def _run(inputs, trace=False, **kw):
    nc = _get_nc()
    in_maps = _make_in_maps(inputs["q"], inputs["k"], inputs["v"],
                            inputs["Wq"], inputs["Wk"], inputs["Wv"], inputs["Wo"])
    res = None
    for attempt in range(3):
        try:
            res = run_bass_kernel_spmd(nc, in_maps, core_ids=list(range(8)),
                                       trace=trace, **kw)
            break
        except Exception:
            if attempt == 2:
                raise
            import time
            time.sleep(2.0)
    out = np.empty((B, N, C), np.float32)
    for c in range(8):
        b, r = c // 4, c % 4
        out[b, NQ * r:NQ * (r + 1), :] = res.results[c]["outT"].T
    return out, res


def kernel(**inputs) -> np.ndarray:
    out, _ = _run(inputs, trace=False)
    return out
